# revision 13
# baseline (speedup 1.0000x reference)
"""Trainium2 Bass kernel for nn_Explainer (gnn_message_passing).

Sparse reformulation: the dense [N,N] mask/adj matrices are never built.
masked_adj[i,j] = A(i,j) * (S(i,j)+S(j,i))/2, with A = scatter of adj_data,
S = scatter of gate values.  Expanding the product gives incidence-pairs
(e', e): each contributes adj_data[e'] * value[e] / 2 * V[src] to output row
`dest`, where (dest,src) is the ordered pair of e' and value[e] is the
edge-MLP gate of e.  Host does index-only preprocessing (bucketing = the
sharding); all arithmetic runs on the 8 NeuronCores.

Per core (1024 dest rows, row-sharded):
  - record table R[n] = [G1'(n) | G2(n) | V(n)] (bf16, 256 elems = 512B rows)
    where G1' = embed@W1e[0:128] + (embed[nodeid]@W1e[256:384] + b1e),
    G2 = embed@W1e[128:256], V = x@Wg0.  Built once via PE.
  - main slot grid [8 strips x 128 rows x 40 slots]: dma_gather R[src],
    MLP logit = w2 . relu(G1'[dest] + G2[src]) (G1'[dest] comes from an
    aligned gather), gates, then Xg = g * V[src]; segment-sum via PE:
    psumT[feat,dest] += Xg^T (matmul with identity rhs).
  - general chunks (overflow + reverse-orientation + final): gather R[src]
    and R[dest], same MLP, scatter via a compare-built onehot rhs.
  - t = relu(h1) @ Wg1 per strip; out[nodeid] = sum g_final * t[src_local]
    via tiny matmuls; AllReduce([1,7]); softmax on device.
"""

import os
import sys

import numpy as np

for _p in ("/opt/trn_rl_repo", "/root/.axon_site/_ro/trn_rl_repo"):
    if os.path.isdir(_p) and _p not in sys.path:
        sys.path.insert(0, _p)

import concourse.bacc as bacc  # noqa: E402
import concourse.bass as bass  # noqa: E402
import concourse.mybir as mybir  # noqa: E402
import concourse.tile as tile  # noqa: E402
from concourse.bass_utils import run_bass_kernel_spmd  # noqa: E402

N = 8192
E = 262144
D = 128   # embed dim
F = 128   # input feature dim
H = 128   # GCN hidden
C = 7     # classes

NCORES = 8
RPC = N // NCORES          # rows per core (1024)
STRIPS = RPC // 128        # 8
SLOTS = 40                 # main-grid slots per dest row
NOVF = 8                   # general chunks for direct-overflow
NREV = 4                   # general chunks for reverse-orientation pairs
# general chunk layout: [0..NOVF-1]=ovf-direct, [NOVF]=final-direct,
# [NOVF+1 .. NOVF+NREV]=reverse, [NOVF+NREV+1]=final-reverse
CH_FIN_D = NOVF
CH_REV0 = NOVF + 1
CH_FIN_R = NOVF + NREV + 1
NGEN = NOVF + NREV + 2     # 14

F32 = mybir.dt.float32
BF16 = mybir.dt.bfloat16
I16 = mybir.dt.int16
AX = mybir.AxisListType
ALU = mybir.AluOpType
ACTF = mybir.ActivationFunctionType

NP_BF16 = mybir.dt.np(BF16)

MAIN_IDX = STRIPS * SLOTS * 128          # 40960 per core
GEN_IDX = NGEN * 128                     # 1792 per core


# --------------------------------------------------------------------------
# host-side index preprocessing (sharding): numpy, indices only
# --------------------------------------------------------------------------

def _pack_idx(flat: np.ndarray) -> np.ndarray:
    """Pack gather indices into the SWDGE layout: [128, n/16] int16 with
    idx i at [i%16, i//16], replicated across the 8 Q7 core groups."""
    n = flat.shape[0]
    assert n % 16 == 0
    a = np.zeros((128, n // 16), np.int16)
    i = np.arange(n)
    v = flat.astype(np.int16)
    for k in range(8):
        a[i % 16 + 16 * k, i // 16] = v
    return a


def _slotify(arr_1024x: np.ndarray, ncols: int) -> np.ndarray:
    """[1024, ncols] (dest-local major) -> [128, STRIPS*ncols] with
    element (p, s*ncols + j) = arr[s*128+p, j]."""
    return (
        arr_1024x.reshape(STRIPS, 128, ncols)
        .transpose(1, 0, 2)
        .reshape(128, STRIPS * ncols)
        .copy()
    )


def _prepare(row, col, nodeid):
    """Build per-core incidence-pair schedules.  Index manipulation only —
    no input *values* are touched here."""
    row = np.asarray(row).astype(np.int64)
    col = np.asarray(col).astype(np.int64)
    nodeid = int(np.asarray(nodeid))

    keep = row != col
    eids = np.nonzero(keep)[0]           # original edge ids
    r = row[eids]
    c = col[eids]
    key = r * N + c
    order = np.argsort(key, kind="stable")
    ks = key[order]
    es = eids[order]                      # edge ids sorted by (row,col)
    uniq, gstart = np.unique(ks, return_index=True)
    gcnt = np.diff(np.append(gstart, len(ks)))
    ngroups = len(uniq)
    gi = (uniq // N).astype(np.int64)     # pair row
    gj = (uniq % N).astype(np.int64)      # pair col
    # reverse group lookup
    rkey = gj * N + gi
    rpos = np.searchsorted(uniq, rkey)
    rpos_c = np.minimum(rpos, ngroups - 1)
    rfound = uniq[rpos_c] == rkey
    rpos = np.where(rfound, rpos_c, -1)

    # incidence-pair lists: columns dest, src, ep (adj_data index), ev (value
    # edge index), typ (0 direct, 1 reverse)
    dests, srcs, eps, evs, typs = [], [], [], [], []

    # direct pairs: cnt==1 fast path
    ones = gcnt == 1
    g1 = np.nonzero(ones)[0]
    e1 = es[gstart[g1]]
    dests.append(gi[g1]); srcs.append(gj[g1])
    eps.append(e1); evs.append(e1)
    typs.append(np.zeros(len(g1), np.int8))
    # duplicate groups: cartesian product
    for g in np.nonzero(~ones)[0]:
        ge = es[gstart[g]: gstart[g] + gcnt[g]]
        ep_, ev_ = np.meshgrid(ge, ge, indexing="ij")
        k = ep_.size
        dests.append(np.full(k, gi[g])); srcs.append(np.full(k, gj[g]))
        eps.append(ep_.ravel()); evs.append(ev_.ravel())
        typs.append(np.zeros(k, np.int8))
    # reverse pairs: e' in (i,j), e in (j,i) -> contributes to w(i,j)
    for g in np.nonzero(rpos >= 0)[0]:
        rg = rpos[g]
        gep = es[gstart[g]: gstart[g] + gcnt[g]]
        gev = es[gstart[rg]: gstart[rg] + gcnt[rg]]
        ep_, ev_ = np.meshgrid(gep, gev, indexing="ij")
        k = ep_.size
        dests.append(np.full(k, gi[g])); srcs.append(np.full(k, gj[g]))
        eps.append(ep_.ravel()); evs.append(ev_.ravel())
        typs.append(np.ones(k, np.int8))

    dest = np.concatenate(dests)
    src = np.concatenate(srcs)
    ep = np.concatenate(eps)
    ev = np.concatenate(evs)
    typ = np.concatenate(typs)

    # final incidences: dest == nodeid, routed to core(src)
    fin = dest == nodeid

    cores = []
    for cid in range(NCORES):
        lo, hi = cid * RPC, (cid + 1) * RPC
        m = (dest >= lo) & (dest < hi)
        md = m & (typ == 0)
        mr = m & (typ == 1)

        # ---- main grid (direct type) ----
        dl = dest[md] - lo
        o2 = np.argsort(dl, kind="stable")
        dl_s = dl[o2]
        src_s = src[md][o2]
        ep_s = ep[md][o2]
        ev_s = ev[md][o2]
        rank = np.arange(len(dl_s)) - np.searchsorted(dl_s, dl_s)
        inmain = rank < SLOTS

        g_src = np.zeros((RPC, SLOTS), np.int64)
        g_ep = np.full((RPC, SLOTS), -1, np.int64)
        g_ev = np.full((RPC, SLOTS), -1, np.int64)
        g_src[dl_s[inmain], rank[inmain]] = src_s[inmain]
        g_ep[dl_s[inmain], rank[inmain]] = ep_s[inmain]
        g_ev[dl_s[inmain], rank[inmain]] = ev_s[inmain]

        # ---- general chunk slots ----
        gen_src = np.zeros(NGEN * 128, np.int64)
        gen_dst = np.zeros(NGEN * 128, np.int64)
        gen_ep = np.full(NGEN * 128, -1, np.int64)
        gen_ev = np.full(NGEN * 128, -1, np.int64)
        gen_dloc = np.zeros(NGEN * 128, np.int64)
        gen_sloc = np.zeros(NGEN * 128, np.int64)

        def fill(c0, nch, d_, s_, ep_, ev_, what):
            cap = nch * 128
            k = len(d_)
            assert k <= cap, f"core {cid}: {what} {k} > budget {cap}"
            sl = slice(c0 * 128, c0 * 128 + k)
            gen_src[sl] = s_
            gen_dst[sl] = d_
            gen_ep[sl] = ep_
            gen_ev[sl] = ev_
            gen_dloc[sl] = d_ - lo
            gen_sloc[sl] = s_ - (s_ >> 10 << 10)

        # overflow of the main grid
        fill(0, NOVF, dest[md][o2][~inmain], src_s[~inmain],
             ep_s[~inmain], ev_s[~inmain], "ovf-direct")
        # reverse pairs with dest on this core
        fill(CH_REV0, NREV, dest[mr], src[mr], ep[mr], ev[mr], "reverse")
        # final incidences routed by src block
        fd = fin & (typ == 0) & (src >= lo) & (src < hi)
        fr = fin & (typ == 1) & (src >= lo) & (src < hi)
        fill(CH_FIN_D, 1, dest[fd], src[fd], ep[fd], ev[fd], "final-direct")
        fill(CH_FIN_R, 1, dest[fr], src[fr], ep[fr], ev[fr], "final-reverse")
        # srcloc only meaningful for final chunks; zero elsewhere
        msk = np.zeros(NGEN * 128, bool)
        msk[CH_FIN_D * 128:(CH_FIN_D + 1) * 128] = True
        msk[CH_FIN_R * 128:(CH_FIN_R + 1) * 128] = True
        gen_sloc = np.where(msk & (gen_ev >= 0), gen_src - lo, 0)

        # gather orderings
        main_src_flat = (
            g_src.reshape(STRIPS, 128, SLOTS).transpose(0, 2, 1).reshape(-1)
        )  # strip-major, slot, then row
        cores.append(dict(
            mainsrc=_pack_idx(main_src_flat),
            alignsrc=_pack_idx(np.arange(lo, hi)),
            gensrc=_pack_idx(gen_src),
            gendst=_pack_idx(gen_dst),
            main_ep=_slotify(g_ep, SLOTS),        # [128, 320] int64 (-1 pad)
            main_ev=_slotify(g_ev, SLOTS),
            gen_ep=gen_ep.reshape(NGEN, 128).T.copy(),
            gen_ev=gen_ev.reshape(NGEN, 128).T.copy(),
            gen_dloc=gen_dloc.reshape(NGEN, 128).T.astype(np.float32).copy(),
            gen_sloc=gen_sloc.reshape(NGEN, 128).T.astype(np.float32).copy(),
        ))
    return cores


def _gather_vals(vals: np.ndarray, idx: np.ndarray, pad: float) -> np.ndarray:
    """Permute an input-value array into slot layout (pure gather; -1 -> pad)."""
    out = np.full(idx.shape, pad, np.float32)
    ok = idx >= 0
    out[ok] = vals[idx[ok]]
    return out


# --------------------------------------------------------------------------
# device kernel
# --------------------------------------------------------------------------

def _build_nc(no_cc=None, no_gather=None, dbg=None):
    if no_cc is None:
        no_cc = bool(int(os.environ.get("K_NO_CC", "0")))
    if no_gather is None:
        no_gather = bool(int(os.environ.get("K_NO_GATHER", "0")))
    if dbg is None:
        dbg = bool(int(os.environ.get("K_DEBUG", "0")))
    nc = bacc.Bacc("TRN2", target_bir_lowering=False, debug=False,
                   enable_asserts=False, num_devices=NCORES)

    def inp(name, shape, dt=F32):
        return nc.dram_tensor(name, list(shape), dt, kind="ExternalInput")

    embT = inp("embT", [128, N])          # embed^T (host permuted)
    xT = inp("xT", [128, N])              # x^T
    W1e = inp("W1e", [384, 64])
    b1e = inp("b1e", [1, 64])
    w2row = inp("w2row", [1, 64])
    b2e = inp("b2e", [1, 1])
    tmp = inp("tmp", [1, 1])
    Wg0 = inp("Wg0", [128, 128])
    Wg1 = inp("Wg1", [128, C])
    emb5 = inp("emb5", [128, 1])
    mainsrc = inp("mainsrc", [128, MAIN_IDX // 16], I16)
    alignsrc = inp("alignsrc", [128, RPC // 16], I16)
    gensrc = inp("gensrc", [128, GEN_IDX // 16], I16)
    gendst = inp("gendst", [128, GEN_IDX // 16], I16)
    m_noise = inp("m_noise", [128, STRIPS * SLOTS])
    m_adjd = inp("m_adjd", [128, STRIPS * SLOTS])
    g_noise = inp("g_noise", [128, NGEN])
    g_adjd = inp("g_adjd", [128, NGEN])
    g_dloc = inp("g_dloc", [128, NGEN])
    g_sloc = inp("g_sloc", [128, NGEN])
    iota1024 = inp("iota1024", [128, RPC])
    ident128 = inp("ident128", [128, 128], BF16)
    ones_inc = inp("ones_inc", [128, 1])
    ones_row = inp("ones_row", [1, 128])

    out_ext = nc.dram_tensor("out", [1, C], F32, kind="ExternalOutput")
    if dbg:
        dbg_arec = nc.dram_tensor("dbg_arec", [128, STRIPS, 256], F32,
                                  kind="ExternalOutput")
        dbg_rec0 = nc.dram_tensor("dbg_rec0", [128, 256], BF16,
                                  kind="ExternalOutput")
        dbg_t = nc.dram_tensor("dbg_t", [128, STRIPS, C], F32,
                               kind="ExternalOutput")
        dbg_gg = nc.dram_tensor("dbg_gg", [128, NGEN], F32,
                                kind="ExternalOutput")
        dbg_la0 = nc.dram_tensor("dbg_la0", [128, SLOTS], F32,
                                 kind="ExternalOutput")
        dbg_g0 = nc.dram_tensor("dbg_g0", [128, SLOTS], F32,
                                kind="ExternalOutput")
        dbg_stg0 = nc.dram_tensor("dbg_stg0", [128, SLOTS, 256], BF16,
                                  kind="ExternalOutput")
        dbg_h1t = nc.dram_tensor("dbg_h1t", [128, STRIPS, 128], F32,
                                 kind="ExternalOutput")

    v = nc.vector
    sc = nc.scalar
    gp = nc.gpsimd
    te = nc.tensor
    sy = nc.sync

    with tile.TileContext(nc) as tc:
        with (
            tc.tile_pool(name="const", bufs=1) as cpool,
            tc.tile_pool(name="big", bufs=1) as bigpool,
            tc.tile_pool(name="stage", bufs=2) as stpool,
            tc.tile_pool(name="work", bufs=2) as wpool,
            tc.tile_pool(name="small", bufs=2) as spool,
            tc.tile_pool(name="psm", bufs=3, space="PSUM") as psm,
            tc.tile_pool(name="pss", bufs=2, space="PSUM") as pss,
            tc.tile_pool(name="psB", bufs=1, space="PSUM") as psB,
            tc.tile_pool(name="dram", bufs=1, space="DRAM") as dpool,
        ):
            # ---------- constants / small inputs into SBUF ----------
            _loadn = [0]

            def load(pool, ap, dt=None, shape=None, tag=None):
                _loadn[0] += 1
                t = pool.tile(list(shape or ap.shape), dt or ap.dtype,
                              tag=tag or f"ld{_loadn[0]}")
                sy.dma_start(out=t[:], in_=ap[:])
                return t

            iota_sb = load(cpool, iota1024)
            id_sb = load(cpool, ident128)
            ones_inc_sb = load(cpool, ones_inc)
            ones_row_sb = load(cpool, ones_row)
            w1_12 = cpool.tile([128, 128], F32, tag="w1_12")
            sy.dma_start(out=w1_12[:, 0:64], in_=W1e[0:128, :])
            sy.dma_start(out=w1_12[:, 64:128], in_=W1e[128:256, :])
            w1_12b = cpool.tile([128, 128], BF16, tag="w1_12b")
            v.tensor_copy(out=w1_12b[:], in_=w1_12[:])
            w1bot = load(cpool, W1e[256:384, :], shape=[128, 64])
            wg0_sb = load(cpool, Wg0)
            wg0b = cpool.tile([128, 128], BF16, tag="wg0b")
            v.tensor_copy(out=wg0b[:], in_=wg0_sb[:])
            wg1_sb = load(cpool, Wg1)
            b1_sb = load(cpool, b1e)
            w2_sb = load(cpool, w2row)
            b2_sb = load(cpool, b2e)
            tmp_sb = load(cpool, tmp)
            emb5_sb = load(cpool, emb5)
            mainsrc_sb = load(cpool, mainsrc)
            alignsrc_sb = load(cpool, alignsrc)
            gensrc_sb = load(cpool, gensrc)
            gendst_sb = load(cpool, gendst)
            mnoise_sb = load(cpool, m_noise)
            madjd_sb = load(cpool, m_adjd)
            gnoise_sb = load(cpool, g_noise)
            gadjd_sb = load(cpool, g_adjd)
            gdloc_sb = load(cpool, g_dloc)
            gsloc_sb = load(cpool, g_sloc)

            # ---------- tiny broadcast helpers (outer products) ----------
            # c = emb5 @ W1e[256:] + b1e ; replicated across partitions
            ps_c = psm.tile([128, 128], F32, tag="ps")
            te.matmul(out=ps_c[0:1, 0:64], lhsT=emb5_sb[:], rhs=w1bot[:],
                      start=True, stop=True)
            cvec = spool.tile([1, 64], F32, tag="cvec")
            v.tensor_add(out=cvec[:], in0=ps_c[0:1, 0:64], in1=b1_sb[:])
            ps_crep = psm.tile([128, 128], F32, tag="ps")
            te.matmul(out=ps_crep[:, 0:64], lhsT=ones_row_sb[:], rhs=cvec[:],
                      start=True, stop=True)
            c_rep = cpool.tile([128, 64], F32, tag="c_rep")
            v.tensor_copy(out=c_rep[:], in_=ps_crep[:, 0:64])

            ps_w2 = psm.tile([128, 128], F32, tag="ps")
            te.matmul(out=ps_w2[:, 0:64], lhsT=ones_row_sb[:], rhs=w2_sb[:],
                      start=True, stop=True)
            w2rep = cpool.tile([128, 64], BF16, tag="w2rep")
            v.tensor_copy(out=w2rep[:], in_=ps_w2[:, 0:64])

            invt = spool.tile([1, 1], F32, tag="invt")
            v.reciprocal(out=invt[:], in_=tmp_sb[:])
            ps_it = psm.tile([128, 128], F32, tag="ps")
            te.matmul(out=ps_it[:, 0:1], lhsT=ones_row_sb[:], rhs=invt[:],
                      start=True, stop=True)
            invt_rep = cpool.tile([128, 1], F32, tag="invt_rep")
            v.tensor_copy(out=invt_rep[:], in_=ps_it[:, 0:1])
            ps_b2 = psm.tile([128, 128], F32, tag="ps")
            te.matmul(out=ps_b2[:, 0:1], lhsT=ones_row_sb[:], rhs=b2_sb[:],
                      start=True, stop=True)
            b2rep = cpool.tile([128, 1], F32, tag="b2rep")
            v.tensor_copy(out=b2rep[:], in_=ps_b2[:, 0:1])

            # ---------- build record table R = [G1' | G2 | V] ----------
            embTb = bigpool.tile([128, N], BF16, tag="embTb")
            gp.dma_start(out=embTb[:], in_=embT[:])   # cast f32->bf16 in DMA
            xTb = bigpool.tile([128, N], BF16, tag="xTb")
            gp.dma_start(out=xTb[:], in_=xT[:])

            R_dram = dpool.tile([N, 256], BF16, tag="R")
            for t in range(N // 128):
                sl = slice(t * 128, (t + 1) * 128)
                ps_g = psm.tile([128, 128], F32, tag="ps")
                te.matmul(out=ps_g[:], lhsT=embTb[:, sl], rhs=w1_12b[:],
                          start=True, stop=True)
                ps_v = psm.tile([128, 128], F32, tag="ps")
                te.matmul(out=ps_v[:], lhsT=xTb[:, sl], rhs=wg0b[:],
                          start=True, stop=True)
                rec = wpool.tile([128, 256], BF16, tag="recbld")
                v.tensor_add(out=rec[:, 0:64], in0=ps_g[:, 0:64], in1=c_rep[:])
                v.tensor_copy(out=rec[:, 64:128], in_=ps_g[:, 64:128])
                v.tensor_copy(out=rec[:, 128:256], in_=ps_v[:])
                sy.dma_start(out=R_dram[sl, :], in_=rec[:])

            # ---------- noise logits eta = log(u) - log(1-u) ----------
            def make_eta(noise_sb, ncols, tag):
                eta = spool.tile([128, ncols], F32, tag=f"eta{tag}")
                t1 = spool.tile([128, ncols], F32, tag=f"etat{tag}")
                sc.activation(out=eta[:], in_=noise_sb[:], func=ACTF.Ln)
                v.tensor_scalar(out=t1[:], in0=noise_sb[:], scalar1=-1.0,
                                scalar2=1.0, op0=ALU.mult, op1=ALU.add)
                sc.activation(out=t1[:], in_=t1[:], func=ACTF.Ln)
                v.tensor_sub(out=eta[:], in0=eta[:], in1=t1[:])
                return eta

            eta_m = make_eta(mnoise_sb, STRIPS * SLOTS, "m")
            eta_g = make_eta(gnoise_sb, NGEN, "g")
            adjh_m = spool.tile([128, STRIPS * SLOTS], F32, tag="adjhm")
            v.tensor_scalar_mul(out=adjh_m[:], in0=madjd_sb[:], scalar1=0.5)
            adjh_g = spool.tile([128, NGEN], F32, tag="adjhg")
            v.tensor_scalar_mul(out=adjh_g[:], in0=gadjd_sb[:], scalar1=0.5)

            def gates(logalpha, eta_slice, adjh_slice, ncols, tag):
                """g = sigmoid((eta + logalpha + b2)/tmp) * adjd/2, f32."""
                gin = spool.tile([128, ncols], F32, tag=f"gin{tag}")
                v.tensor_add(out=gin[:], in0=eta_slice, in1=logalpha)
                v.tensor_scalar(out=gin[:], in0=gin[:],
                                scalar1=b2rep[:, 0:1], scalar2=invt_rep[:, 0:1],
                                op0=ALU.add, op1=ALU.mult)
                sc.activation(out=gin[:], in_=gin[:], func=ACTF.Sigmoid)
                g = spool.tile([128, ncols], F32, tag=f"g{tag}")
                v.tensor_mul(out=g[:], in0=gin[:], in1=adjh_slice)
                return g

            def mlp_logalpha(stage, aligned_in1, nch, tag, reverse_rng=None):
                """stage[:, :, 0:64] (+= -> relu -> *w2 -> reduce)."""
                la = spool.tile([128, nch], F32, tag=f"la{tag}")
                if reverse_rng is None:
                    v.tensor_add(out=stage[:, :, 0:64],
                                 in0=stage[:, :, 64:128], in1=aligned_in1)
                sc.activation(out=stage[:, :, 0:64], in_=stage[:, :, 0:64],
                              func=ACTF.Relu)
                v.tensor_tensor(out=stage[:, :, 0:64], in0=stage[:, :, 0:64],
                                in1=w2rep[:].unsqueeze(1)
                                .to_broadcast([128, nch, 64]),
                                op=ALU.mult)
                v.reduce_sum(out=la[:], in_=stage[:, :, 0:64], axis=AX.X)
                return la

            # ---------- general chunks ----------
            grec = wpool.tile([128, NGEN, 256], BF16, tag="grec")
            gdst = wpool.tile([128, NGEN, 256], BF16, tag="gdst")
            if no_gather:
                v.memset(grec[:], 0.001)
                v.memset(gdst[:], 0.001)
            else:
                gp.dma_gather(grec[:], R_dram[:], gensrc_sb[:], GEN_IDX,
                              GEN_IDX, 256, single_packet=False)
                gp.dma_gather(gdst[:], R_dram[:], gendst_sb[:], GEN_IDX,
                              GEN_IDX, 256, single_packet=False)
            # direct chunks [0..NOVF]: h = G1'[dest] + G2[src]
            v.tensor_add(out=grec[:, 0:CH_FIN_D + 1, 0:64],
                         in0=gdst[:, 0:CH_FIN_D + 1, 0:64],
                         in1=grec[:, 0:CH_FIN_D + 1, 64:128])
            # reverse chunks [CH_REV0..]: h = G1'[src] + G2[dest]
            v.tensor_add(out=grec[:, CH_REV0:NGEN, 0:64],
                         in0=grec[:, CH_REV0:NGEN, 0:64],
                         in1=gdst[:, CH_REV0:NGEN, 64:128])
            la_g = mlp_logalpha(grec, None, NGEN, "g", reverse_rng=True)
            gg = gates(la_g[:], eta_g[:], adjh_g[:], NGEN, "g")
            ggb = spool.tile([128, NGEN], BF16, tag="ggb")
            v.tensor_copy(out=ggb[:], in_=gg[:])
            v.tensor_tensor(out=grec[:, :, 128:256], in0=grec[:, :, 128:256],
                            in1=ggb[:].unsqueeze(2)
                            .to_broadcast([128, NGEN, 128]),
                            op=ALU.mult)

            ps_ovf = psB.tile([128, RPC], F32)
            sc_chunks = [ch for ch in range(NGEN)
                         if ch not in (CH_FIN_D, CH_FIN_R)]
            for k, ch in enumerate(sc_chunks):
                s_t = wpool.tile([128, RPC], BF16, tag="S")
                v.tensor_tensor(out=s_t[:],
                                in0=gdloc_sb[:, ch:ch + 1]
                                .to_broadcast([128, RPC]),
                                in1=iota_sb[:], op=ALU.is_equal)
                st = k == 0
                sp = k == len(sc_chunks) - 1
                te.matmul(out=ps_ovf[:, 0:512], lhsT=grec[:, ch, 128:256],
                          rhs=s_t[:, 0:512], start=st, stop=sp)
                te.matmul(out=ps_ovf[:, 512:1024], lhsT=grec[:, ch, 128:256],
                          rhs=s_t[:, 512:1024], start=st, stop=sp)

            ovf_sb = bigpool.tile([128, RPC], F32, tag="ovf_sb")
            v.tensor_copy(out=ovf_sb[:], in_=ps_ovf[:])

            # ---------- aligned records (G1' for this core's rows) ----------
            arec = bigpool.tile([128, STRIPS, 256], BF16, tag="arec")
            if no_gather:
                v.memset(arec[:], 0.001)
            else:
                gp.dma_gather(arec[:], R_dram[:], alignsrc_sb[:], RPC, RPC,
                              256, single_packet=False)

            if dbg:
                arec_f = wpool.tile([128, STRIPS, 256], F32, tag="arecf")
                v.tensor_copy(out=arec_f[:], in_=arec[:])
                sy.dma_start(out=dbg_arec[:], in_=arec_f[:])
                rec0_sb = wpool.tile([128, 256], BF16, tag="rec0")
                sy.dma_start(out=rec0_sb[:], in_=R_dram[0:128, :])
                sy.dma_start(out=dbg_rec0[:], in_=rec0_sb[:])
                sy.dma_start(out=dbg_gg[:], in_=gg[:])

            # ---------- main strips ----------
            t_all = cpool.tile([128, STRIPS, C], F32, tag="t_all")
            for s in range(STRIPS):
                stg = stpool.tile([128, SLOTS, 256], BF16, tag="stg")
                c0 = s * (SLOTS * 128 // 16)
                if no_gather:
                    v.memset(stg[:], 0.001)
                else:
                    gp.dma_gather(stg[:], R_dram[:],
                                  mainsrc_sb[:, c0:c0 + SLOTS * 128 // 16],
                                  SLOTS * 128, SLOTS * 128, 256,
                                  single_packet=False)
                la = mlp_logalpha(
                    stg, arec[:, s, 0:64].unsqueeze(1)
                    .to_broadcast([128, SLOTS, 64]), SLOTS, "m")
                gm = gates(la[:], eta_m[:, s * SLOTS:(s + 1) * SLOTS],
                           adjh_m[:, s * SLOTS:(s + 1) * SLOTS], SLOTS, "m")
                if dbg and s == 0:
                    sy.dma_start(out=dbg_la0[:], in_=la[:])
                    sy.dma_start(out=dbg_g0[:], in_=gm[:])
                    sy.dma_start(out=dbg_stg0[:], in_=stg[:])
                gmb = spool.tile([128, SLOTS], BF16, tag="gmb")
                v.tensor_copy(out=gmb[:], in_=gm[:])
                v.tensor_tensor(out=stg[:, :, 128:256], in0=stg[:, :, 128:256],
                                in1=gmb[:].unsqueeze(2)
                                .to_broadcast([128, SLOTS, 128]),
                                op=ALU.mult)
                ps_s = pss.tile([128, 128], F32, tag="strip")
                for j in range(SLOTS):
                    te.matmul(out=ps_s[:], lhsT=stg[:, j, 128:256],
                              rhs=id_sb[:], start=(j == 0),
                              stop=(j == SLOTS - 1))
                h1t = wpool.tile([128, 128], F32, tag="h1t")
                v.tensor_add(out=h1t[:], in0=ps_s[:],
                             in1=ovf_sb[:, s * 128:(s + 1) * 128])
                sc.activation(out=h1t[:], in_=h1t[:], func=ACTF.Relu)
                if dbg:
                    sy.dma_start(out=dbg_h1t[:, s, :], in_=h1t[:])
                ps_t = psm.tile([128, 128], F32, tag="ps")
                te.matmul(out=ps_t[:, 0:C], lhsT=h1t[:], rhs=wg1_sb[:],
                          start=True, stop=True)
                v.tensor_copy(out=t_all[:, s, :], in_=ps_t[:, 0:C])

            if dbg:
                sy.dma_start(out=dbg_t[:], in_=t_all[:])

            # ---------- final dot: out7 = sum g_fin * t[src_local] ----------
            ps7 = psB.tile([1, C], F32)
            for s in range(STRIPS):
                ps_w = psm.tile([128, 128], F32, tag="ps")
                for k, ch in enumerate((CH_FIN_D, CH_FIN_R)):
                    oh = wpool.tile([128, 128], F32, tag="oh")
                    v.tensor_tensor(out=oh[:],
                                    in0=gsloc_sb[:, ch:ch + 1]
                                    .to_broadcast([128, 128]),
                                    in1=iota_sb[:, s * 128:(s + 1) * 128],
                                    op=ALU.is_equal)
                    v.tensor_scalar(out=oh[:], in0=oh[:],
                                    scalar1=gg[:, ch:ch + 1], scalar2=None,
                                    op0=ALU.mult)
                    te.matmul(out=ps_w[:, 0:1], lhsT=oh[:], rhs=ones_inc_sb[:],
                              start=(k == 0), stop=(k == 1))
                wsel = spool.tile([128, 1], F32, tag="wsel_sb")
                v.tensor_copy(out=wsel[:], in_=ps_w[:, 0:1])
                te.matmul(out=ps7[:], lhsT=wsel[:], rhs=t_all[:, s, :],
                          start=(s == 0), stop=(s == STRIPS - 1))

            # ---------- AllReduce + softmax ----------
            sb7 = spool.tile([1, C], F32, tag="sb7")
            v.tensor_copy(out=sb7[:], in_=ps7[:])
            cc_in = dpool.tile([1, C], F32, tag="cc_in")
            cc_out = dpool.tile([1, C], F32, tag="cc_out")
            sy.dma_start(out=cc_in[:], in_=sb7[:])
            if no_cc:
                sy.dma_start(out=cc_out[:], in_=cc_in[:])
            else:
                gp.collective_compute(
                    "AllReduce", ALU.add,
                    replica_groups=[list(range(NCORES))],
                    ins=[cc_in[:].opt()], outs=[cc_out[:].opt()])
            red = spool.tile([1, C], F32, tag="red")
            sy.dma_start(out=red[:], in_=cc_out[:])
            mx = spool.tile([1, 1], F32, tag="mx")
            v.reduce_max(out=mx[:], in_=red[:], axis=AX.X)
            v.tensor_scalar(out=red[:], in0=red[:], scalar1=mx[:, 0:1],
                            scalar2=None, op0=ALU.subtract)
            sc.activation(out=red[:], in_=red[:], func=ACTF.Exp)
            sm = spool.tile([1, 1], F32, tag="sm")
            v.reduce_sum(out=sm[:], in_=red[:], axis=AX.X)
            rc = spool.tile([1, 1], F32, tag="rc")
            v.reciprocal(out=rc[:], in_=sm[:])
            v.tensor_scalar(out=red[:], in0=red[:], scalar1=rc[:, 0:1],
                            scalar2=None, op0=ALU.mult)
            sy.dma_start(out=out_ext[:], in_=red[:])

    nc.compile()
    return nc


_NC_CACHE = {}


def _get_nc():
    if "nc" not in _NC_CACHE:
        _NC_CACHE["nc"] = _build_nc()
    return _NC_CACHE["nc"]


# --------------------------------------------------------------------------
# entry point
# --------------------------------------------------------------------------

def _make_inmaps(x, embed, adj_data, noise, tmp, W1e, b1e, W2e, b2e, Wg0,
                 Wg1, row, col, nodeid):
    x = np.asarray(x, np.float32)
    embed = np.asarray(embed, np.float32)
    adj_data = np.asarray(adj_data, np.float32)
    noise = np.asarray(noise, np.float32)
    nodeid_i = int(np.asarray(nodeid))

    cores = _prepare(row, col, nodeid_i)

    shared = dict(
        embT=np.ascontiguousarray(embed.T),
        xT=np.ascontiguousarray(x.T),
        W1e=np.asarray(W1e, np.float32),
        b1e=np.asarray(b1e, np.float32).reshape(1, 64),
        w2row=np.asarray(W2e, np.float32).reshape(1, 64),
        b2e=np.asarray(b2e, np.float32).reshape(1, 1),
        tmp=np.asarray(tmp, np.float32).reshape(1, 1),
        Wg0=np.asarray(Wg0, np.float32),
        Wg1=np.asarray(Wg1, np.float32),
        emb5=np.ascontiguousarray(embed[nodeid_i].reshape(128, 1)),
        iota1024=np.tile(np.arange(RPC, dtype=np.float32), (128, 1)),
        ident128=np.eye(128, dtype=np.float32).astype(NP_BF16),
        ones_inc=np.ones((128, 1), np.float32),
        ones_row=np.ones((1, 128), np.float32),
    )

    in_maps = []
    for cid in range(NCORES):
        cd = cores[cid]
        in_maps.append(dict(
            shared,
            mainsrc=cd["mainsrc"],
            alignsrc=cd["alignsrc"],
            gensrc=cd["gensrc"],
            gendst=cd["gendst"],
            m_noise=_gather_vals(noise, cd["main_ev"], 0.5),
            m_adjd=_gather_vals(adj_data, cd["main_ep"], 0.0),
            g_noise=_gather_vals(noise, cd["gen_ev"], 0.5),
            g_adjd=_gather_vals(adj_data, cd["gen_ep"], 0.0),
            g_dloc=cd["gen_dloc"],
            g_sloc=cd["gen_sloc"],
        ))

    return in_maps


def kernel(_want_trace=False, **inputs):
    in_maps = _make_inmaps(**inputs)
    nc = _get_nc()
    res = run_bass_kernel_spmd(nc, in_maps, core_ids=list(range(NCORES)),
                               trace=_want_trace)
    global _LAST_RES
    _LAST_RES = res
    out = np.asarray(res.results[0]["out"]).reshape(C).astype(np.float32)
    if _want_trace:
        return out, res
    return out
